# revision 9
# baseline (speedup 1.0000x reference)
"""Trainium2 Bass kernel for nn_Block_70712341562194 (dense transformer block).

Sharding: token-parallel within batch. Core c handles batch b=c//2 and query-token
half (c%2) of 512 tokens. K/V are computed for all 1024 tokens of the batch
(duplicated within the pair) so no cross-core communication is needed.

Layout strategy: activations transposed on-chip via PE so every matmul contracts
over the partition dim. Attention runs twice on PE (natural [n,m] scores for the
softmax/attn output, transposed [m,n] scores for the attn@v contraction) which
avoids all large transposes of the attention matrix.

LN affine params and the residual gammas are folded into the weights on the host.
MLP / proj / V run in bf16 (their contribution to x_out is scaled by gamma=1e-5);
the Q/K/scores path runs in fp32r for attn-output accuracy.
"""

import numpy as np
import ml_dtypes

B, N, C, H, HD, HID = 4, 1024, 1024, 16, 64, 4096
NCORES = 8
P = 128
NO = 512  # own query tokens per core
EPS = 1e-5
BF16NP = ml_dtypes.bfloat16

_CACHE = {}


def _build_nc():
    import concourse.bacc as bacc
    import concourse.mybir as mybir
    import concourse.tile as tile
    from concourse.masks import make_identity

    F32, BF16, F32R = mybir.dt.float32, mybir.dt.bfloat16, mybir.dt.float32r
    AF = mybir.ActivationFunctionType
    ALU = mybir.AluOpType

    nc = bacc.Bacc("TRN2", target_bir_lowering=False, debug=False)

    x_d = nc.dram_tensor("x", [N, C], F32, kind="ExternalInput")
    wq_d = nc.dram_tensor("wq", [C, C], F32R, kind="ExternalInput")
    wk_d = nc.dram_tensor("wk", [C, C], F32R, kind="ExternalInput")
    wv_d = nc.dram_tensor("wv", [C, C], F32R, kind="ExternalInput")
    bq_d = nc.dram_tensor("bq", [P, 8], F32, kind="ExternalInput")
    bk_d = nc.dram_tensor("bk", [P, 8], F32, kind="ExternalInput")
    bv_d = nc.dram_tensor("bv", [1, C], F32R, kind="ExternalInput")
    pw_d = nc.dram_tensor("pw", [C, C], BF16, kind="ExternalInput")
    pb_d = nc.dram_tensor("pb", [1, C], BF16, kind="ExternalInput")
    f1w_d = nc.dram_tensor("f1w", [32, C, P], BF16, kind="ExternalInput")
    f1b_d = nc.dram_tensor("f1b", [P, 32], F32, kind="ExternalInput")
    f2w_d = nc.dram_tensor("f2w", [HID, C], BF16, kind="ExternalInput")
    f2b_d = nc.dram_tensor("f2b", [1, C], BF16, kind="ExternalInput")

    attn_d = nc.dram_tensor("attn_o", [H, NO, N], F32, kind="ExternalOutput")
    xo_d = nc.dram_tensor("xo", [NO, C], F32, kind="ExternalOutput")

    # The host rolls the batch's tokens so this core's 512 query tokens are
    # always rows [0, NO); attention columns come out rolled and are un-rolled
    # on assembly. xown = rolled x rows [0, NO) for the residual adds.
    xown_d = nc.dram_tensor("xown", [NO, C], F32, kind="ExternalInput")

    def lnstats(sb, cst, xt, eps_t, xhat):
        st6 = sb.tile([P, 2, 6], F32, tag="st6")
        st2 = sb.tile([P, 2], F32, tag="st2")
        nc.vector.bn_stats(st6[:, 0, :], xt[:, 0:512])
        nc.vector.bn_stats(st6[:, 1, :], xt[:, 512:1024])
        nc.vector.bn_aggr(st2[:], st6[:])
        ve = sb.tile([P, 1], F32, tag="ve")
        nc.vector.tensor_scalar(out=ve[:], in0=st2[:, 1:2], scalar1=EPS,
                                scalar2=None, op0=ALU.add)
        s0 = sb.tile([P, 1], F32, tag="s0")
        nc.scalar.activation(s0[:], ve[:], AF.Sqrt, bias=eps_t[:], scale=1.0)
        r0 = sb.tile([P, 1], F32, tag="r0")
        nc.vector.reciprocal(r0[:], s0[:])
        # one Newton step for rsqrt accuracy: rs = r0*(1.5 - 0.5*ve*r0^2)
        t1 = sb.tile([P, 1], F32, tag="t1")
        nc.vector.tensor_tensor(out=t1[:], in0=r0[:], in1=r0[:], op=ALU.mult)
        nc.vector.tensor_tensor(out=t1[:], in0=t1[:], in1=ve[:], op=ALU.mult)
        nc.vector.tensor_scalar(out=t1[:], in0=t1[:], scalar1=-0.5, scalar2=1.5,
                                op0=ALU.mult, op1=ALU.add)
        rs = sb.tile([P, 1], F32, tag="rs")
        nc.vector.tensor_tensor(out=rs[:], in0=r0[:], in1=t1[:], op=ALU.mult)
        nc.vector.tensor_scalar(out=xhat[:], in0=xt[:], scalar1=st2[:, 0:1],
                                scalar2=rs[:], op0=ALU.subtract, op1=ALU.mult)

    with tile.TileContext(nc) as tc:
        with tc.tile_pool(name="cst", bufs=1) as cst, \
             tc.tile_pool(name="res", bufs=1) as res, \
             tc.tile_pool(name="sb", bufs=8) as sb, \
             tc.tile_pool(name="xld", bufs=3) as xld, \
             tc.tile_pool(name="xh", bufs=5) as xh, \
             tc.tile_pool(name="wst", bufs=3) as wst, \
             tc.tile_pool(name="ex", bufs=4) as ex, \
             tc.tile_pool(name="an", bufs=2) as an, \
             tc.tile_pool(name="psA", bufs=4, space="PSUM") as psA, \
             tc.tile_pool(name="psB", bufs=2, space="PSUM") as psB, \
             tc.tile_pool(name="psC", bufs=2, space="PSUM") as psC, \
             tc.tile_pool(name="dr", bufs=1, space="DRAM") as dr:

            ident = cst.tile([P, P], F32)
            make_identity(nc, ident)
            eps_t = cst.tile([P, 1], F32)
            nc.vector.memset(eps_t[:], 0.0)
            ones1 = cst.tile([1, P], BF16)
            nc.vector.memset(ones1[:], 1.0)

            bq_sb = cst.tile([P, 8], F32, tag="bq")
            bk_sb = cst.tile([P, 8], F32, tag="bk")
            bv_sb = cst.tile([1, C], F32R, tag="bv")
            pb_sb = cst.tile([1, C], BF16, tag="pb")
            f1b_sb = cst.tile([P, 32], F32, tag="f1b")
            f2b_sb = cst.tile([1, C], BF16, tag="f2b")
            nc.sync.dma_start(out=bq_sb[:], in_=bq_d[:, :])
            nc.sync.dma_start(out=bk_sb[:], in_=bk_d[:, :])
            nc.sync.dma_start(out=bv_sb[:], in_=bv_d[:, :])
            nc.sync.dma_start(out=pb_sb[:], in_=pb_d[:, :])
            nc.sync.dma_start(out=f1b_sb[:], in_=f1b_d[:, :])
            nc.sync.dma_start(out=f2b_sb[:], in_=f2b_d[:, :])

            n1T = res.tile([P, 8, N], F32R, tag="n1T")
            qT = res.tile([P, 8, NO], F32R, tag="qT")
            kT = res.tile([P, 8, N], F32R, tag="kThT")
            v_sb = res.tile([P, 8, N], BF16, tag="vout1")

            # ---------------- Phase A: LN1 + transpose to n1T ----------------
            for g in range(2):
                xh_g = []
                for j in range(4):
                    nt = 4 * g + j
                    xt = xld.tile([P, C], F32, tag="x")
                    nc.sync.dma_start(out=xt[:], in_=x_d[nt * P:(nt + 1) * P, :])
                    xhat = xh.tile([P, C], F32, tag="xh")
                    lnstats(sb, cst, xt, eps_t, xhat)
                    xh_g.append(xhat)
                for ct in range(8):
                    tp = psB.tile([P, 512], F32, tag="tr")
                    for j in range(4):
                        nc.tensor.transpose(tp[:, j * P:(j + 1) * P],
                                            xh_g[j][:, ct * P:(ct + 1) * P], ident[:])
                    nc.vector.tensor_copy(n1T[:, ct, g * 512:(g + 1) * 512], tp[:])

            # ---------------- Phase B: QKV projections ----------------
            # Q: own tokens are columns [0, NO) of n1T (host rolls tokens).
            for g in range(2):
                pq = [psA.tile([P, 512], F32, tag="mm", name=f"pq{g}_{j}") for j in range(4)]
                for ct in range(8):
                    wch = wst.tile([P, 512], F32R, tag="wf32")
                    nc.sync.dma_start(out=wch[:], in_=wq_d[ct * P:(ct + 1) * P,
                                                          g * 512:(g + 1) * 512])
                    for j in range(4):
                        nc.tensor.matmul(pq[j][:], wch[:, j * P:(j + 1) * P],
                                         n1T[:, ct, 0:NO],
                                         start=(ct == 0), stop=(ct == 7))
                for j in range(4):
                    ot = 4 * g + j
                    nc.vector.tensor_scalar(out=qT[:, ot, :], in0=pq[j][:],
                                            scalar1=bq_sb[:, ot:ot + 1], scalar2=None,
                                            op0=ALU.add)
            # K: all tokens
            for nch in range(2):
                for g in range(2):
                    pk = [psA.tile([P, 512], F32, tag="mm", name=f"pk{nch}_{g}_{j}") for j in range(4)]
                    for ct in range(8):
                        wch = wst.tile([P, 512], F32R, tag="wf32")
                        nc.sync.dma_start(out=wch[:], in_=wk_d[ct * P:(ct + 1) * P,
                                                              g * 512:(g + 1) * 512])
                        for j in range(4):
                            nc.tensor.matmul(pk[j][:], wch[:, j * P:(j + 1) * P],
                                             n1T[:, ct, nch * 512:(nch + 1) * 512],
                                             start=(ct == 0), stop=(ct == 7))
                    for j in range(4):
                        ot = 4 * g + j
                        nc.vector.tensor_scalar(out=kT[:, ot, nch * 512:(nch + 1) * 512],
                                                in0=pk[j][:], scalar1=bk_sb[:, ot:ot + 1],
                                                scalar2=None, op0=ALU.add)
            # V: natural layout [m, o], bf16, bias via K=1 matmul
            onesr_f = cst.tile([1, P], F32, tag="onesrf")
            nc.vector.memset(onesr_f[:], 1.0)
            onesr = cst.tile([1, P], F32R, tag="onesr")
            nc.vector.tensor_copy(onesr[:], onesr_f[:])
            for och in range(2):
                for mg in range(2):
                    pv = [psA.tile([P, 512], F32, tag="mm", name=f"pv{och}_{mg}_{j}") for j in range(4)]
                    for ct in range(8):
                        wch = wst.tile([P, 512], F32R, tag="wf32")
                        nc.sync.dma_start(out=wch[:], in_=wv_d[ct * P:(ct + 1) * P,
                                                               och * 512:(och + 1) * 512])
                        for j in range(4):
                            mt = 4 * mg + j
                            nc.tensor.matmul(pv[j][:], n1T[:, ct, mt * P:(mt + 1) * P],
                                             wch[:],
                                             start=(ct == 0), stop=False)
                    for j in range(4):
                        mt = 4 * mg + j
                        nc.tensor.matmul(pv[j][:], onesr[:],
                                         bv_sb[0:1, och * 512:(och + 1) * 512],
                                         start=False, stop=True)
                        nc.vector.tensor_copy(v_sb[:, mt, och * 512:(och + 1) * 512], pv[j][:])

            # ---------------- Phase C: attention ----------------
            aoT = res.tile([P, 8, NO], BF16, tag="aon2T")
            recip = res.tile([P, 4, H], F32, tag="recip")
            avp = None
            for h in range(H):
                h2, off = h // 2, 64 * (h % 2)
                # transposed scores -> exp -> av accumulation.  Emission is
                # software-pipelined (sc[mt+1] ahead of av[mt]) so the in-order
                # PE stream isn't blocked on ACT's exp.
                if h % 2 == 0:
                    avp = psC.tile([P, NO], F32, tag="av")
                ets = [None] * 8

                def emit_sc(mt):
                    sc = psA.tile([P, 512], F32, tag="mm", name=f"sc{h}_{mt}")
                    nc.tensor.matmul(sc[:], kT[off:off + 64, h2, mt * P:(mt + 1) * P],
                                     qT[off:off + 64, h2, :],
                                     start=True, stop=True, tile_position=(off, 0))
                    et = ex.tile([P, 512], BF16, tag="expT", name=f"et{h}_{mt}")
                    nc.scalar.activation(et[:], sc[:], AF.Exp, scale=0.125)
                    ets[mt] = et

                def emit_av(mt):
                    nc.tensor.matmul(avp[off:off + 64, :], v_sb[:, mt, 64 * h:64 * h + 64],
                                     ets[mt][:], start=(mt == 0), stop=(mt == 7),
                                     tile_position=(0, off))

                emit_sc(0)
                emit_sc(1)
                for mt in range(8):
                    if mt + 2 < 8:
                        emit_sc(mt + 2)
                    emit_av(mt)
                # natural scores -> exp(+accum) -> normalize -> attn output
                for nt in range(4):
                    a_t = an.tile([P, N], F32, tag="anat")
                    s01 = sb.tile([P, 2], F32, tag="s01")
                    for mch in range(2):
                        nsp = psA.tile([P, 512], F32, tag="mm", name=f"nsp{h}_{nt}_{mch}")
                        nc.tensor.matmul(nsp[:], qT[off:off + 64, h2, nt * P:(nt + 1) * P],
                                         kT[off:off + 64, h2, mch * 512:(mch + 1) * 512],
                                         start=True, stop=True, tile_position=(off, 0))
                        nc.scalar.activation(a_t[:, mch * 512:(mch + 1) * 512], nsp[:],
                                             AF.Exp, scale=0.125,
                                             accum_out=s01[:, mch:mch + 1])
                    ssum = sb.tile([P, 1], F32, tag="ssum")
                    nc.vector.tensor_tensor(out=ssum[:], in0=s01[:, 0:1], in1=s01[:, 1:2],
                                            op=ALU.add)
                    nc.vector.reciprocal(recip[:, nt, h:h + 1], ssum[:])
                    nc.vector.tensor_scalar(out=a_t[:], in0=a_t[:],
                                            scalar1=recip[:, nt, h:h + 1], scalar2=None,
                                            op0=ALU.mult)
                    nc.sync.dma_start(out=attn_d[h, nt * P:(nt + 1) * P, :], in_=a_t[:])
                if h % 2 == 1:
                    nc.vector.tensor_copy(aoT[:, h2, :], avp[:])

            # reciprocal rows transposed + broadcast via DRAM round-trip
            rtp = psC.tile([16, NO], F32, tag="av")
            for nt in range(4):
                nc.tensor.transpose(rtp[:, nt * P:(nt + 1) * P], recip[:, nt, :], ident[:])
            rT = sb.tile([16, NO], BF16, tag="rT")
            nc.vector.tensor_copy(rT[:], rtp[:])
            rdram = dr.tile([16, NO], BF16)
            nc.sync.dma_start(out=rdram[:], in_=rT[:])
            for p in range(8):
                bct = ex.tile([P, NO], BF16, tag="bct")
                nc.sync.dma_start(out=bct[0:64, :],
                                  in_=rdram[2 * p:2 * p + 1, :].broadcast_to([64, NO]))
                nc.sync.dma_start(out=bct[64:128, :],
                                  in_=rdram[2 * p + 1:2 * p + 2, :].broadcast_to([64, NO]))
                nc.vector.tensor_tensor(out=aoT[:, p, :], in0=aoT[:, p, :], in1=bct[:],
                                        op=ALU.mult)

            # ---------------- Phase D: proj + residual ----------------
            out1 = res.tile([P, 4, C], F32, tag="vout1")
            for och in range(2):
                pp = [psA.tile([P, 512], F32, tag="mm", name=f"pp{och}_{j}") for j in range(4)]
                for p in range(8):
                    wch = wst.tile([P, 512], BF16, tag="wb16")
                    nc.sync.dma_start(out=wch[:], in_=pw_d[p * P:(p + 1) * P,
                                                           och * 512:(och + 1) * 512])
                    for nt in range(4):
                        nc.tensor.matmul(pp[nt][:], aoT[:, p, nt * P:(nt + 1) * P],
                                         wch[:], start=(p == 0), stop=False)
                for nt in range(4):
                    nc.tensor.matmul(pp[nt][:], ones1[:],
                                     pb_sb[0:1, och * 512:(och + 1) * 512],
                                     start=False, stop=True)
                    xot = xld.tile([P, 512], F32, tag="xo2")
                    nc.sync.dma_start(out=xot[:], in_=xown_d[nt * P:(nt + 1) * P,
                                                             och * 512:(och + 1) * 512])
                    nc.vector.tensor_tensor(out=out1[:, nt, och * 512:(och + 1) * 512],
                                            in0=pp[nt][:], in1=xot[:], op=ALU.add)

            # ---------------- Phase E: LN2 + transpose ----------------
            n2T = res.tile([P, 8, NO], BF16, tag="aon2T")
            xh2 = []
            for nt in range(4):
                xhat = xh.tile([P, C], F32, tag="xh")
                lnstats(sb, cst, out1[:, nt, :], eps_t, xhat)
                xh2.append(xhat)
            for ct in range(8):
                tp = psB.tile([P, 512], F32, tag="tr")
                for j in range(4):
                    nc.tensor.transpose(tp[:, j * P:(j + 1) * P],
                                        xh2[j][:, ct * P:(ct + 1) * P], ident[:])
                nc.vector.tensor_copy(n2T[:, ct, :], tp[:])

            # ---------------- Phase F: fc1 + gelu ----------------
            hT = res.tile([P, 32, NO], BF16, tag="kThT")
            for ht in range(32):
                f1t = wst.tile([P, 8, P], BF16, tag="f1w")
                nc.sync.dma_start(out=f1t[:], in_=f1w_d[ht].rearrange(
                    "(ct p) f -> p ct f", p=P))
                pf = psB.tile([P, 512], F32, tag="tr")
                for ct in range(8):
                    nc.tensor.matmul(pf[:], f1t[:, ct, :], n2T[:, ct, :],
                                     start=(ct == 0), stop=(ct == 7))
                nc.scalar.activation(hT[:, ht, :], pf[:], AF.Gelu,
                                     bias=f1b_sb[:, ht:ht + 1], scale=1.0)

            # ---------------- Phase G: fc2 + residual ----------------
            for och in range(2):
                pts = [psA.tile([P, 512], F32, tag="mm", name=f"pts{och}_{j}") for j in range(4)]
                for ht in range(32):
                    wch = wst.tile([P, 512], BF16, tag="wb16")
                    nc.sync.dma_start(out=wch[:], in_=f2w_d[ht * P:(ht + 1) * P,
                                                            och * 512:(och + 1) * 512])
                    for nt in range(4):
                        nc.tensor.matmul(pts[nt][:], hT[:, ht, nt * P:(nt + 1) * P],
                                         wch[:], start=(ht == 0), stop=False)
                for nt in range(4):
                    nc.tensor.matmul(pts[nt][:], ones1[:],
                                     f2b_sb[0:1, och * 512:(och + 1) * 512],
                                     start=False, stop=True)
                    xo_t = xld.tile([P, 512], F32, tag="xo2")
                    nc.vector.tensor_tensor(out=xo_t[:], in0=pts[nt][:],
                                            in1=out1[:, nt, och * 512:(och + 1) * 512],
                                            op=ALU.add)
                    nc.sync.dma_start(out=xo_d[nt * P:(nt + 1) * P,
                                               och * 512:(och + 1) * 512], in_=xo_t[:])

    nc.compile()
    return nc


def _host_prep(inputs):
    f32 = lambda a: np.ascontiguousarray(np.asarray(a, dtype=np.float32))
    x = f32(inputs["x"])
    qkv_w = f32(inputs["qkv_w"])
    g1, b1 = f32(inputs["ln1_g"]), f32(inputs["ln1_b"])
    g2, b2 = f32(inputs["ln2_g"]), f32(inputs["ln2_b"])
    gamma1, gamma2 = f32(inputs["gamma1"]), f32(inputs["gamma2"])
    proj_w, proj_b = f32(inputs["proj_w"]), f32(inputs["proj_b"])
    fc1_w, fc1_b = f32(inputs["fc1_w"]), f32(inputs["fc1_b"])
    fc2_w, fc2_b = f32(inputs["fc2_w"]), f32(inputs["fc2_b"])

    bf = lambda a: np.ascontiguousarray(a.astype(BF16NP))
    com = {}
    com["wq"] = np.ascontiguousarray((qkv_w[0:C] * g1[None, :]).T)
    com["wk"] = np.ascontiguousarray((qkv_w[C:2 * C] * g1[None, :]).T)
    com["wv"] = np.ascontiguousarray((qkv_w[2 * C:3 * C] * g1[None, :]).T)
    com["bq"] = np.ascontiguousarray((qkv_w[0:C] @ b1).reshape(8, P).T)
    com["bk"] = np.ascontiguousarray((qkv_w[C:2 * C] @ b1).reshape(8, P).T)
    com["bv"] = np.ascontiguousarray((qkv_w[2 * C:3 * C] @ b1).reshape(1, C))
    com["pw"] = bf((proj_w * gamma1[:, None]).T)
    com["pb"] = bf((proj_b * gamma1).reshape(1, C))
    f1w = fc1_w.T * g2[:, None]                       # [C, HID]
    com["f1w"] = bf(f1w.reshape(C, 32, P).transpose(1, 0, 2))
    com["f1b"] = np.ascontiguousarray((fc1_b + fc1_w @ b2).reshape(32, P).T)
    com["f2w"] = bf(fc2_w.T * gamma2[None, :])
    com["f2b"] = bf((fc2_b * gamma2).reshape(1, C))
    return x, com


def kernel(**inputs):
    from concourse import bass_utils

    x, com = _host_prep(inputs)

    if "nc" not in _CACHE:
        _CACHE["nc"] = _build_nc()
    nc = _CACHE["nc"]

    in_maps = []
    for c in range(NCORES):
        b, halfc = c // 2, c % 2
        tok0 = halfc * NO
        xb = x[b]
        # roll tokens so own 512 tokens are rows [0, 512)
        xr = np.ascontiguousarray(np.roll(xb, -tok0, axis=0))
        m = dict(com)
        m["x"] = xr
        m["xown"] = np.ascontiguousarray(xr[0:NO])
        in_maps.append(m)

    res = bass_utils.run_bass_kernel_spmd(
        nc, in_maps, core_ids=list(range(NCORES)),
        trace=bool(_CACHE.get("trace", False)),
    )
    _CACHE["last_results"] = res

    x_out = np.empty((B, N, C), np.float32)
    attn = np.empty((B, H, N, N), np.float32)
    for c in range(NCORES):
        b, halfc = c // 2, c % 2
        tok0 = halfc * NO
        r = res.results[c]
        x_out[b, tok0:tok0 + NO] = r["xo"]
        # attn rows are own tokens (true order); columns are rolled by tok0
        attn[b, :, tok0:tok0 + NO, :] = np.roll(r["attn_o"], tok0, axis=2)
    return x_out, attn
